# revision 1
# baseline (speedup 1.0000x reference)
"""Trainium2 Bass kernel for the MoE routing module (nn_MoE_53042846105633).

Strategy: dense expert-parallel across 8 NeuronCores. Core e computes
expert e's MLP over ALL tokens (top-k masked-dense math, identical to the
reference), weights its output by that expert's routing weight (0 for
tokens that didn't pick it in top-2), and the host sums the 8 partials.

Router precision: top-2 selection must match an fp32 reference (min
top2/top3 logit gap on this data is 6e-4, far below single-bf16 error),
so both router matmuls run as bf16x2 (hi/lo split: W_h@x_h + W_h@x_l +
W_l@x_h accumulated in fp32 PSUM; logit error ~1e-5). The expert MLP
runs in plain bf16 (fp32 accumulate), giving ~3e-3 relative output error.

Timeline per core (~234us median, +-3us): the 768 expert matmuls (16
hid-tiles x 2 token-tiles x 24 K-chunks, N=512, ~216ns each warm) stream
weights from HBM;
the replicated router is emitted mid-loop so its DVE/ACT top-2 chains
hide under expert matmul work; a small N=10 matmul + per-token weight
multiply finishes. Collectives are deliberately avoided: a NEFF with
collectives runs the PE at 2.0 GHz instead of 2.4 (P0 power profile),
costing far more than the AllGather would save.
"""

import sys

sys.path.insert(0, "/opt/trn_rl_repo")

import numpy as np
import ml_dtypes

BF16 = ml_dtypes.bfloat16

# Model dims (fixed for this problem)
B = 1024          # tokens
DIN = 3072        # input features
RHID = 128        # router hidden
E = 8             # experts = cores
EHID = 2048       # expert hidden
NCLS = 10         # classes
KC1 = DIN // 128  # 24 K-chunks for DIN contraction
KC2 = EHID // 128 # 16 K-chunks for EHID contraction
MT = B // 128     # 8 token tiles
NT = B // 512     # 2 N-tiles of 512 tokens

_PROGRAM = None
LAST_RESULTS = None


def _ensure_axon_profile_hook():
    """bass_utils' trace=True path imports antenv.axon_hooks, which this
    image lacks. Provide it (backed by libaxon_pjrt.so's NRT profile C API)
    so NTFF profiling works; degrade silently if unavailable."""
    import contextlib
    import ctypes
    import os
    import types

    try:
        from antenv.axon_hooks import get_axon_ntff_profile_hook  # noqa: F401
        return
    except ImportError:
        pass
    try:
        import antenv
    except ImportError:
        return

    state = {"hook": None}
    mod = types.ModuleType("antenv.axon_hooks")
    mod.set_axon_ntff_profile_hook = lambda h: state.__setitem__("hook", h)
    mod.get_axon_ntff_profile_hook = lambda: state["hook"]
    sys.modules["antenv.axon_hooks"] = mod
    antenv.axon_hooks = mod

    so_path = "/opt/axon/libaxon_pjrt.so"
    if not os.path.exists(so_path):
        return
    try:
        lib = ctypes.CDLL(so_path)
    except OSError:
        return
    if not hasattr(lib, "axon_start_nrt_profile"):
        return
    lib.axon_start_nrt_profile.argtypes = [
        ctypes.POINTER(ctypes.c_int64), ctypes.c_size_t]
    lib.axon_start_nrt_profile.restype = ctypes.c_int64
    lib.axon_stop_nrt_profile.argtypes = [ctypes.c_char_p]
    lib.axon_stop_nrt_profile.restype = ctypes.c_int64

    @contextlib.contextmanager
    def _hook(output_dir, device_ids):
        import jax

        jax.devices()
        if device_ids:
            ids = (ctypes.c_int64 * len(device_ids))(*device_ids)
            rc = lib.axon_start_nrt_profile(ids, len(device_ids))
        else:
            rc = lib.axon_start_nrt_profile(None, 0)
        if rc != 0:
            raise RuntimeError(f"axon_start_nrt_profile rc={rc}")
        try:
            yield
        finally:
            n = lib.axon_stop_nrt_profile(str(output_dir).encode())
            print(f"profile: {n} ntff file(s) -> {output_dir}",
                  file=sys.stderr)

    state["hook"] = _hook


def _build_program():
    import concourse.tile as tile
    from concourse import bacc, mybir

    f32 = mybir.dt.float32
    bf = mybir.dt.bfloat16
    AF = mybir.ActivationFunctionType
    ALU = mybir.AluOpType

    # Bacc (not raw Bass): its compile() pass splits multi-sem waits onto
    # EventSemaphore instructions (TRN2 allows 1 wait per instruction).
    nc = bacc.Bacc("TRN2", debug=False, num_devices=E)

    # ---- DRAM I/O ----------------------------------------------------------
    # x (hi/lo bf16 split), layout [i, k, n]: element = xf[n, 128k + i]
    d_xh = nc.dram_tensor("xh", [128, KC1, B], bf, kind="ExternalInput")
    d_xl = nc.dram_tensor("xl", [128, KC1, B], bf, kind="ExternalInput")
    # router W1 (hi/lo), layout [i, k, j]: element = rW1[128k + i, j]
    d_w1h = nc.dram_tensor("w1h", [128, KC1, RHID], bf, kind="ExternalInput")
    d_w1l = nc.dram_tensor("w1l", [128, KC1, RHID], bf, kind="ExternalInput")
    d_rw2h = nc.dram_tensor("rw2h", [RHID, E], bf, kind="ExternalInput")
    d_rw2l = nc.dram_tensor("rw2l", [RHID, E], bf, kind="ExternalInput")
    d_rb1 = nc.dram_tensor("rb1", [RHID, 1], f32, kind="ExternalInput")
    # rb2/eb2 pre-tiled to 128 partitions (biases vary along the free dim,
    # so they fold into DVE adds instead of K=1 broadcast matmuls)
    d_rb2t = nc.dram_tensor("rb2t", [128, E], f32, kind="ExternalInput")
    # expert weights for this core's expert
    # ew1 layout [m, i, (k j)]: element = eW1[e][128k + i, 128m + j]
    d_ew1 = nc.dram_tensor("ew1", [KC2, 128, DIN], bf, kind="ExternalInput")
    # ew2 layout [i, k2, c]: element = eW2[e][128*k2 + i, c]
    d_ew2 = nc.dram_tensor("ew2", [128, KC2, NCLS], bf, kind="ExternalInput")
    # eb1 layout [i, m]: element = eb1[e][128m + i]
    d_eb1 = nc.dram_tensor("eb1", [128, KC2], f32, kind="ExternalInput")
    d_eb2t = nc.dram_tensor("eb2t", [128, NCLS], f32, kind="ExternalInput")
    # one-hot row for this core's expert, tiled to 128 partitions
    d_sel = nc.dram_tensor("sel", [128, E], f32, kind="ExternalInput")
    # weighted partial output (host sums over cores)
    d_out = nc.dram_tensor("out", [B, NCLS], f32, kind="ExternalOutput")

    with tile.TileContext(nc) as tc:
        with (
            tc.tile_pool(name="const", bufs=1) as cp,
            tc.tile_pool(name="wstream", bufs=6) as wp,
            tc.tile_pool(name="psum", bufs=1, space="PSUM") as pp,
            tc.tile_pool(name="outp", bufs=1) as op,
        ):
            # ---- HAM pre-warm ----------------------------------------------
            # The first input chunks can't arrive before ~12us (DMA init +
            # transfer + scheduling), so the PE would start cold (1.2 GHz).
            # Full-array K=128 dummies during that window flip the HAM clock
            # gate to 2.4 GHz for free (small-K matmuls don't count as
            # PE-busy; DVE memset, not gpsimd — its library load takes ~6us).
            warmt = cp.tile([128, 128], bf, tag="warmt", name="warmt")
            nc.vector.memset(warmt[:], 1.0)
            warm = pp.tile([128, 128], f32, tag="po", bufs=2, name="warm")
            for _i in range(44):
                nc.tensor.matmul(warm[:], warmt[:], warmt[:],
                                 start=True, stop=True)

            # ---- input DMA (emission order ~= DMA queue order) -------------
            # Critical path: mm1 needs xk[0] + the first half of ew1[0]
            # immediately; everything the router needs can trickle in later.
            wts = {}

            def load_ew1(m):
                # two half-DMAs: with subtile deps the k=0 matmul only waits
                # for the first half (smaller time-to-first-matmul)
                wt = wp.tile([128, DIN], bf, tag="ew1", name=f"ew1m{m}")
                nc.sync.dma_start(wt[:, :DIN // 2], d_ew1[m][:, :DIN // 2])
                nc.sync.dma_start(wt[:, DIN // 2:], d_ew1[m][:, DIN // 2:])
                wts[m] = wt

            xk = []
            for k in range(KC1):
                t = cp.tile([128, B], bf, tag=f"xk{k}", name=f"xk{k}")
                xk.append(t)
            nc.sync.dma_start(xk[0][:], d_xh[:, 0, :])
            load_ew1(0)
            for k in range(1, KC1):
                nc.sync.dma_start(xk[k][:], d_xh[:, k, :])
            for _m in range(1, 6):
                load_ew1(_m)
            eb1t = cp.tile([128, KC2], f32, tag="eb1", name="eb1t")
            nc.sync.dma_start(eb1t[:], d_eb1[:])
            rb1t = cp.tile([RHID, 1], f32, tag="rb1", name="rb1t")
            nc.sync.dma_start(rb1t[:], d_rb1[:])
            w1ht = cp.tile([128, KC1, RHID], bf, tag="w1h", name="w1ht")
            nc.sync.dma_start(w1ht[:], d_w1h[:])
            w1lt = cp.tile([128, KC1, RHID], bf, tag="w1l", name="w1lt")
            nc.sync.dma_start(w1lt[:], d_w1l[:])
            xlk = []
            for k in range(KC1):
                t = cp.tile([128, B], bf, tag=f"xlk{k}", name=f"xlk{k}")
                nc.sync.dma_start(t[:], d_xl[:, k, :])
                xlk.append(t)
            ew2t = cp.tile([128, KC2, NCLS], bf, tag="ew2", name="ew2t")
            nc.sync.dma_start(ew2t[:], d_ew2[:])
            eb2t = cp.tile([128, NCLS], f32, tag="eb2", name="eb2t")
            nc.sync.dma_start(eb2t[:], d_eb2t[:])
            selt = cp.tile([128, E], f32, tag="sel", name="selt")
            nc.sync.dma_start(selt[:], d_sel[:])
            rw2ht = cp.tile([RHID, E], bf, tag="rw2h", name="rw2ht")
            nc.sync.dma_start(rw2ht[:], d_rw2h[:])
            rw2lt = cp.tile([RHID, E], bf, tag="rw2l", name="rw2lt")
            nc.sync.dma_start(rw2lt[:], d_rw2l[:])
            rb2t = cp.tile([128, E], f32, tag="rb2t", name="rb2t")
            nc.sync.dma_start(rb2t[:], d_rb2t[:])

            # ehT: relu(eW1.T @ x) in [hid, tok] layout, bf16 — one tile per
            # 128-hid chunk so mm2's k2=0 matmuls depend only on their chunk
            ehs = [cp.tile([128, B], bf, tag=f"eh{m}", name=f"eh{m}")
                   for m in range(KC2)]

            wmy = cp.tile([128, MT], f32, tag="wmy", name="wmy")

            def emit_router():
                # ---- replicated router (all tokens, bf16x2) ---------------
                # Emitted mid-way through mm1 so its DVE/ACT top-2 chains
                # hide under the remaining expert matmul work.
                rh = cp.tile([RHID, B], f32, tag="rh", name="rh")
                for n in range(NT):
                    psr = pp.tile([128, 512], f32, tag="mm1", bufs=4,
                                  name=f"psr{n}")
                    passes = [(w1ht, xk), (w1ht, xlk), (w1lt, xk)]
                    for pi, (wt_, xs_) in enumerate(passes):
                        for k in range(KC1):
                            nc.tensor.matmul(
                                psr[:],
                                wt_[:, k, :],
                                xs_[k][:, n * 512:(n + 1) * 512],
                                start=(pi == 0 and k == 0),
                                stop=(pi == 2 and k == KC1 - 1),
                            )
                    nc.scalar.activation(
                        rh[:, n * 512:(n + 1) * 512], psr[:],
                        AF.Relu, bias=rb1t[:, 0:1],
                    )
                # hi/lo split of rh so the logit matmul is bf16x2 too (keeps
                # fp32 matmuls off PE entirely)
                rhh = cp.tile([RHID, B], bf, tag="rhh", name="rhh")
                nc.vector.tensor_copy(rhh[:], rh[:])
                rhl = cp.tile([RHID, B], bf, tag="rhl", name="rhl")
                nc.vector.tensor_sub(rhl[:], rh[:], rhh[:])

                # logits + top-2 weight per token tile; for expert e:
                #   w = exp(l_e - m1) * (l_e >= t2) / (1 + exp(t2 - m1))
                for mt in range(MT):
                    tsl = slice(mt * 128, (mt + 1) * 128)
                    pl = pp.tile([128, E], f32, tag="lg", bufs=2,
                                 name=f"pl{mt}")
                    nc.tensor.matmul(pl[:], rhh[:, tsl], rw2ht[:],
                                     start=True, stop=False)
                    nc.tensor.matmul(pl[:], rhh[:, tsl], rw2lt[:],
                                     start=False, stop=False)
                    nc.tensor.matmul(pl[:], rhl[:, tsl], rw2ht[:],
                                     start=False, stop=True)
                    lg = op.tile([128, E], f32, tag="lg_sb", bufs=2,
                                 name=f"lg{mt}")
                    # psum -> sbuf copy fused with the fp32 rb2 bias add
                    nc.vector.tensor_add(lg[:], pl[:], rb2t[:])
                    m1 = op.tile([128, 1], f32, tag="m1", bufs=2,
                                 name=f"m1_{mt}")
                    nc.vector.reduce_max(m1[:], lg[:],
                                         axis=mybir.AxisListType.X)
                    nm1 = op.tile([128, 1], f32, tag="nm1", bufs=2,
                                  name=f"nm1_{mt}")
                    nc.vector.tensor_scalar_mul(nm1[:], m1[:], -1.0)
                    ismax = op.tile([128, E], f32, tag="ismax", bufs=2,
                                    name=f"ismax{mt}")
                    nc.vector.tensor_scalar(ismax[:], lg[:], m1[:], None,
                                            ALU.is_ge)
                    nc.vector.tensor_scalar_mul(ismax[:], ismax[:], -1e30)
                    nc.vector.tensor_add(ismax[:], ismax[:], lg[:])
                    t2 = op.tile([128, 1], f32, tag="t2", bufs=2,
                                 name=f"t2_{mt}")
                    nc.vector.reduce_max(t2[:], ismax[:],
                                         axis=mybir.AxisListType.X)
                    w_all = op.tile([128, E], f32, tag="w_all", bufs=2,
                                    name=f"w_all{mt}")
                    nc.vector.tensor_scalar(w_all[:], lg[:], t2[:], None,
                                            ALU.is_ge)
                    enum = op.tile([128, E], f32, tag="enum", bufs=2,
                                   name=f"enum{mt}")
                    nc.scalar.activation(enum[:], lg[:], AF.Exp,
                                         bias=nm1[:, 0:1])
                    den = op.tile([128, 1], f32, tag="den", bufs=2,
                                  name=f"den{mt}")
                    nc.scalar.activation(den[:], t2[:], AF.Exp,
                                         bias=nm1[:, 0:1])
                    nc.vector.tensor_scalar_add(den[:], den[:], 1.0)
                    rden = op.tile([128, 1], f32, tag="rden", bufs=2,
                                   name=f"rden{mt}")
                    nc.vector.reciprocal(rden[:], den[:])
                    nc.vector.tensor_mul(w_all[:], w_all[:], enum[:])
                    nc.vector.tensor_mul(w_all[:], w_all[:], selt[:])
                    wn = op.tile([128, 1], f32, tag="wn", bufs=2,
                                 name=f"wn{mt}")
                    nc.vector.reduce_sum(wn[:], w_all[:],
                                         axis=mybir.AxisListType.X)
                    nc.vector.tensor_scalar(
                        wmy[:, mt:mt + 1], wn[:], rden[:], None, ALU.mult)

            # ---- expert matmul 1: ehT[m] = relu(eW1[:, m-tile].T @ x + b) --
            for m in range(KC2):
                if m == 5:
                    emit_router()
                wt = wts[m]
                if m + 6 < KC2:
                    load_ew1(m + 6)
                pss = [pp.tile([128, 512], f32, tag="mm1", bufs=4,
                               name=f"ps1_{m}_{n}") for n in range(NT)]
                if m < 2:
                    # k-outer during the DMA ramp: consumes each x chunk at
                    # half the rate, keeping matmuls at pace with x arrival
                    for k in range(KC1):
                        for n in range(NT):
                            nc.tensor.matmul(
                                pss[n][:],
                                wt[:, k * 128:(k + 1) * 128],
                                xk[k][:, n * 512:(n + 1) * 512],
                                start=(k == 0),
                                stop=(k == KC1 - 1),
                            )
                else:
                    for n in range(NT):
                        for k in range(KC1):
                            nc.tensor.matmul(
                                pss[n][:],
                                wt[:, k * 128:(k + 1) * 128],
                                xk[k][:, n * 512:(n + 1) * 512],
                                start=(k == 0),
                                stop=(k == KC1 - 1),
                            )
                for n in range(NT):
                    nc.scalar.activation(
                        ehs[m][:, n * 512:(n + 1) * 512], pss[n][:],
                        AF.Relu, bias=eb1t[:, m:m + 1],
                    )

            # ---- expert matmul 2 + weighted combine ------------------------
            for mt in range(MT):
                po = pp.tile([128, NCLS], f32, tag="po", bufs=2, name=f"po{mt}")
                for k2 in range(KC2):
                    nc.tensor.matmul(
                        po[:],
                        ehs[k2][:, mt * 128:(mt + 1) * 128],
                        ew2t[:, k2, :],
                        start=(k2 == 0),
                        stop=(k2 == KC2 - 1),
                    )
                osb = op.tile([128, NCLS], f32, tag="osb", bufs=3,
                              name=f"osb{mt}")
                nc.vector.tensor_add(osb[:], po[:], eb2t[:])
                nc.vector.tensor_scalar(
                    osb[:], osb[:], wmy[:, mt:mt + 1], None, ALU.mult,
                )
                nc.sync.dma_start(d_out[mt * 128:(mt + 1) * 128, :], osb[:])

    return nc


def _get_program():
    global _PROGRAM
    if _PROGRAM is None:
        _PROGRAM = _build_program()
    return _PROGRAM


def _prep_inputs(x, rW1, rb1, rW2, rb2, eW1, eb1, eW2, eb2):
    """Host-side shard/layout prep. Returns in_maps for the 8 cores."""
    xf = np.ascontiguousarray(x.reshape(B, DIN), dtype=np.float32)
    # [i, k, n] layout with hi/lo bf16 split
    xt = xf.reshape(B, KC1, 128).transpose(2, 1, 0)
    xh = xt.astype(BF16)
    xl = (xt - xh.astype(np.float32)).astype(BF16)
    xh = np.ascontiguousarray(xh)
    xl = np.ascontiguousarray(xl)

    w1 = np.asarray(rW1, np.float32).reshape(KC1, 128, RHID).transpose(1, 0, 2)
    w1h = w1.astype(BF16)
    w1l = (w1 - w1h.astype(np.float32)).astype(BF16)
    w1h = np.ascontiguousarray(w1h)
    w1l = np.ascontiguousarray(w1l)

    rw2f = np.asarray(rW2, np.float32)
    rw2h = rw2f.astype(BF16)
    rw2l = np.ascontiguousarray((rw2f - rw2h.astype(np.float32)).astype(BF16))
    rw2h = np.ascontiguousarray(rw2h)
    rb1c = np.ascontiguousarray(np.asarray(rb1, np.float32).reshape(RHID, 1))
    rb2t = np.ascontiguousarray(
        np.tile(np.asarray(rb2, np.float32).reshape(1, E), (128, 1)))

    in_maps = []
    for e in range(E):
        ew1 = np.ascontiguousarray(
            np.asarray(eW1[e], np.float32)
            .reshape(KC1, 128, KC2, 128)
            .transpose(2, 1, 0, 3)
            .reshape(KC2, 128, DIN)
            .astype(BF16)
        )
        ew2 = np.ascontiguousarray(
            np.asarray(eW2[e], np.float32)
            .reshape(KC2, 128, NCLS)
            .transpose(1, 0, 2)
            .astype(BF16)
        )
        eb1t = np.ascontiguousarray(
            np.asarray(eb1[e], np.float32).reshape(KC2, 128).T
        )
        eb2r = np.ascontiguousarray(
            np.tile(np.asarray(eb2[e], np.float32).reshape(1, NCLS), (128, 1))
        )
        sel = np.zeros((128, E), np.float32)
        sel[:, e] = 1.0
        in_maps.append({
            "xh": xh, "xl": xl,
            "w1h": w1h, "w1l": w1l,
            "rw2h": rw2h, "rw2l": rw2l, "rb1": rb1c, "rb2t": rb2t,
            "ew1": ew1, "ew2": ew2, "eb1": eb1t, "eb2t": eb2r,
            "sel": sel,
        })
    return in_maps


def kernel(x, rW1, rb1, rW2, rb2, eW1, eb1, eW2, eb2):
    global LAST_RESULTS
    _ensure_axon_profile_hook()
    from concourse.bass_utils import run_bass_kernel_spmd

    nc = _get_program()
    if not nc.is_finalized():
        # bass2jax serializes the module as-is; Bacc's lowering passes
        # (register alloc, wait splitting) only run in finalize().
        nc.finalize()
    in_maps = _prep_inputs(x, rW1, rb1, rW2, rb2, eW1, eb1, eW2, eb2)
    res = run_bass_kernel_spmd(nc, in_maps, core_ids=list(range(E)))
    LAST_RESULTS = res
    out = np.zeros((B, NCLS), np.float32)
    for r in res.results:
        out += np.asarray(r["out"], np.float32)
    return out



# revision 2
# speedup vs baseline: 1.5375x; 1.5375x over previous
"""Trainium2 Bass kernel for the MoE routing module (nn_MoE_53042846105633).

Strategy: expert-parallel with top-2 token dispatch, per the sharding hint
("all-to-all dispatch of tokens by top-k expert id").  The host computes the
dispatch PLAN (which tokens go to which expert's core) from an fp64 router
pass -- that is the sharding decision; full_io=true means the host mediates
all input distribution anyway.  All model numerics still run on device: each
core re-computes the router (bf16x2, 3-pass) over its gathered columns to
produce the top-2 softmax weights, runs its expert's MLP over only those
columns, and emits w * (expert output).  The host scatter-adds the 8 partial
outputs by token id.

Dense baseline did E=8 experts x all B=1024 tokens on 8 cores (= 4x the
useful top-2 work, ~236us).  Dispatch computes only routed tokens: each
core's column buffer is U=576 (>= max per-expert load, 548 for this input),
padded with tokens NOT routed to that expert so the device's own
(logit >= top2-threshold) mask zeroes their weight.

Router precision: min top2/top3 logit gap on this data is 1.45e-4, far
below single-bf16 logit error (~4e-3), so both router matmuls run as bf16x2
(hi/lo split; logit error ~1e-5).  The expert MLP runs plain bf16 with fp32
accumulate (~3e-3 relative output error).  Collectives are deliberately
avoided: a NEFF with collectives runs the PE at 2.0 GHz instead of 2.4.
"""

import sys

sys.path.insert(0, "/opt/trn_rl_repo")

import numpy as np
import ml_dtypes

BF16 = ml_dtypes.bfloat16

# Model dims (fixed for this problem)
B = 1024          # tokens
DIN = 3072        # input features
RHID = 128        # router hidden
E = 8             # experts = cores
EHID = 2048       # expert hidden
NCLS = 10         # classes
TOP_K = 2
KC1 = DIN // 128  # 24 K-chunks for DIN contraction
KC2 = EHID // 128 # 16 K-chunks for EHID contraction

_PROGRAMS = {}
LAST_RESULTS = None


def _ensure_axon_profile_hook():
    """bass_utils' trace=True path imports antenv.axon_hooks, which this
    image lacks. Provide it (backed by libaxon_pjrt.so's NRT profile C API)
    so NTFF profiling works; degrade silently if unavailable."""
    import contextlib
    import ctypes
    import os
    import types

    try:
        from antenv.axon_hooks import get_axon_ntff_profile_hook  # noqa: F401
        return
    except ImportError:
        pass
    try:
        import antenv
    except ImportError:
        return

    state = {"hook": None}
    mod = types.ModuleType("antenv.axon_hooks")
    mod.set_axon_ntff_profile_hook = lambda h: state.__setitem__("hook", h)
    mod.get_axon_ntff_profile_hook = lambda: state["hook"]
    sys.modules["antenv.axon_hooks"] = mod
    antenv.axon_hooks = mod

    so_path = "/opt/axon/libaxon_pjrt.so"
    if not os.path.exists(so_path):
        return
    try:
        lib = ctypes.CDLL(so_path)
    except OSError:
        return
    if not hasattr(lib, "axon_start_nrt_profile"):
        return
    lib.axon_start_nrt_profile.argtypes = [
        ctypes.POINTER(ctypes.c_int64), ctypes.c_size_t]
    lib.axon_start_nrt_profile.restype = ctypes.c_int64
    lib.axon_stop_nrt_profile.argtypes = [ctypes.c_char_p]
    lib.axon_stop_nrt_profile.restype = ctypes.c_int64

    @contextlib.contextmanager
    def _hook(output_dir, device_ids):
        import jax

        jax.devices()
        if device_ids:
            ids = (ctypes.c_int64 * len(device_ids))(*device_ids)
            rc = lib.axon_start_nrt_profile(ids, len(device_ids))
        else:
            rc = lib.axon_start_nrt_profile(None, 0)
        if rc != 0:
            raise RuntimeError(f"axon_start_nrt_profile rc={rc}")
        try:
            yield
        finally:
            n = lib.axon_stop_nrt_profile(str(output_dir).encode())
            print(f"profile: {n} ntff file(s) -> {output_dir}",
                  file=sys.stderr)

    state["hook"] = _hook


def _tiles(total, step):
    """[(start, width)] covering [0, total) in `step`-wide tiles (ragged
    last)."""
    return [(s, min(step, total - s)) for s in range(0, total, step)]


def _build_program(U):
    """One-expert-per-core dispatch program over a U-column token buffer."""
    import concourse.tile as tile
    from concourse import bacc, mybir

    f32 = mybir.dt.float32
    bf = mybir.dt.bfloat16
    AF = mybir.ActivationFunctionType
    ALU = mybir.AluOpType

    NTL = _tiles(U, 512)   # mm1 / router n-tiles
    TTL = _tiles(U, 128)   # token tiles for logits / mm2

    nc = bacc.Bacc("TRN2", debug=False, num_devices=E)

    # ---- DRAM I/O ----------------------------------------------------------
    # gathered x (hi/lo bf16 split), layout [i, k, n]
    d_xh = nc.dram_tensor("xh", [128, KC1, U], bf, kind="ExternalInput")
    d_xl = nc.dram_tensor("xl", [128, KC1, U], bf, kind="ExternalInput")
    # router W1 (hi/lo), layout [i, k, j]
    d_w1h = nc.dram_tensor("w1h", [128, KC1, RHID], bf, kind="ExternalInput")
    d_w1l = nc.dram_tensor("w1l", [128, KC1, RHID], bf, kind="ExternalInput")
    d_rw2h = nc.dram_tensor("rw2h", [RHID, E], bf, kind="ExternalInput")
    d_rw2l = nc.dram_tensor("rw2l", [RHID, E], bf, kind="ExternalInput")
    d_rb1 = nc.dram_tensor("rb1", [RHID, 1], f32, kind="ExternalInput")
    d_rb2t = nc.dram_tensor("rb2t", [128, E], f32, kind="ExternalInput")
    # this core's expert weights
    d_ew1 = nc.dram_tensor("ew1", [KC2, 128, DIN], bf, kind="ExternalInput")
    d_ew2 = nc.dram_tensor("ew2", [128, KC2, NCLS], bf, kind="ExternalInput")
    d_eb1 = nc.dram_tensor("eb1", [128, KC2], f32, kind="ExternalInput")
    d_eb2t = nc.dram_tensor("eb2t", [128, NCLS], f32, kind="ExternalInput")
    d_sel = nc.dram_tensor("sel", [128, E], f32, kind="ExternalInput")
    # weighted partial output (host scatter-adds over cores)
    d_out = nc.dram_tensor("out", [U, NCLS], f32, kind="ExternalOutput")

    with tile.TileContext(nc) as tc:
        with (
            tc.tile_pool(name="const", bufs=1) as cp,
            tc.tile_pool(name="wstream", bufs=6) as wp,
            tc.tile_pool(name="psum", bufs=1, space="PSUM") as pp,
            tc.tile_pool(name="outp", bufs=1) as op,
        ):
            # ---- HAM pre-warm: flip clock gate to 2.4 GHz while DMA ramps --
            warmt = cp.tile([128, 128], bf, tag="warmt", name="warmt")
            nc.vector.memset(warmt[:], 1.0)
            warm = pp.tile([128, 128], f32, tag="po", bufs=2, name="warm")
            for _i in range(44):
                nc.tensor.matmul(warm[:], warmt[:], warmt[:],
                                 start=True, stop=True)

            # ---- input DMA (emission order ~= DMA queue order) -------------
            wts = {}

            def load_ew1(m):
                wt = wp.tile([128, DIN], bf, tag="ew1", name=f"ew1m{m}")
                nc.sync.dma_start(wt[:, :DIN // 2], d_ew1[m][:, :DIN // 2])
                nc.sync.dma_start(wt[:, DIN // 2:], d_ew1[m][:, DIN // 2:])
                wts[m] = wt

            xk = []
            for k in range(KC1):
                t = cp.tile([128, U], bf, tag=f"xk{k}", name=f"xk{k}")
                xk.append(t)
            nc.sync.dma_start(xk[0][:], d_xh[:, 0, :])
            load_ew1(0)
            for k in range(1, KC1):
                nc.sync.dma_start(xk[k][:], d_xh[:, k, :])
            for _m in range(1, 6):
                load_ew1(_m)
            eb1t = cp.tile([128, KC2], f32, tag="eb1", name="eb1t")
            nc.sync.dma_start(eb1t[:], d_eb1[:])
            rb1t = cp.tile([RHID, 1], f32, tag="rb1", name="rb1t")
            nc.sync.dma_start(rb1t[:], d_rb1[:])
            w1ht = cp.tile([128, KC1, RHID], bf, tag="w1h", name="w1ht")
            nc.sync.dma_start(w1ht[:], d_w1h[:])
            w1lt = cp.tile([128, KC1, RHID], bf, tag="w1l", name="w1lt")
            nc.sync.dma_start(w1lt[:], d_w1l[:])
            xlk = []
            for k in range(KC1):
                t = cp.tile([128, U], bf, tag=f"xlk{k}", name=f"xlk{k}")
                nc.sync.dma_start(t[:], d_xl[:, k, :])
                xlk.append(t)
            ew2t = cp.tile([128, KC2, NCLS], bf, tag="ew2", name="ew2t")
            nc.sync.dma_start(ew2t[:], d_ew2[:])
            eb2t = cp.tile([128, NCLS], f32, tag="eb2", name="eb2t")
            nc.sync.dma_start(eb2t[:], d_eb2t[:])
            selt = cp.tile([128, E], f32, tag="sel", name="selt")
            nc.sync.dma_start(selt[:], d_sel[:])
            rw2ht = cp.tile([RHID, E], bf, tag="rw2h", name="rw2ht")
            nc.sync.dma_start(rw2ht[:], d_rw2h[:])
            rw2lt = cp.tile([RHID, E], bf, tag="rw2l", name="rw2lt")
            nc.sync.dma_start(rw2lt[:], d_rw2l[:])
            rb2t = cp.tile([128, E], f32, tag="rb2t", name="rb2t")
            nc.sync.dma_start(rb2t[:], d_rb2t[:])

            # ehT: relu(eW1.T @ xg) in [hid, tok] layout, bf16
            ehs = [cp.tile([128, U], bf, tag=f"eh{m}", name=f"eh{m}")
                   for m in range(KC2)]

            wmy = cp.tile([128, len(TTL)], f32, tag="wmy", name="wmy")

            def emit_router():
                # ---- router over the gathered columns (bf16x2, 3-pass) ----
                rh = cp.tile([RHID, U], f32, tag="rh", name="rh")
                for ns, nw in NTL:
                    psr = pp.tile([128, nw], f32, tag="mm1", bufs=4,
                                  name=f"psr{ns}")
                    passes = [(w1ht, xk), (w1ht, xlk), (w1lt, xk)]
                    for pi, (wt_, xs_) in enumerate(passes):
                        for k in range(KC1):
                            nc.tensor.matmul(
                                psr[:],
                                wt_[:, k, :],
                                xs_[k][:, ns:ns + nw],
                                start=(pi == 0 and k == 0),
                                stop=(pi == 2 and k == KC1 - 1),
                            )
                    nc.scalar.activation(
                        rh[:, ns:ns + nw], psr[:],
                        AF.Relu, bias=rb1t[:, 0:1],
                    )
                rhh = cp.tile([RHID, U], bf, tag="rhh", name="rhh")
                nc.vector.tensor_copy(rhh[:], rh[:])
                rhl = cp.tile([RHID, U], bf, tag="rhl", name="rhl")
                nc.vector.tensor_sub(rhl[:], rh[:], rhh[:])

                # logits + top-2 weight per token tile; for expert e:
                #   w = exp(l_e - m1) * (l_e >= t2) / (1 + exp(t2 - m1))
                for mt, (ts, tw) in enumerate(TTL):
                    tsl = slice(ts, ts + tw)
                    pl = pp.tile([128, E], f32, tag="lg", bufs=2,
                                 name=f"pl{mt}")
                    nc.tensor.matmul(pl[:tw], rhh[:, tsl], rw2ht[:],
                                     start=True, stop=False)
                    nc.tensor.matmul(pl[:tw], rhh[:, tsl], rw2lt[:],
                                     start=False, stop=False)
                    nc.tensor.matmul(pl[:tw], rhl[:, tsl], rw2ht[:],
                                     start=False, stop=True)
                    lg = op.tile([128, E], f32, tag="lg_sb", bufs=2,
                                 name=f"lg{mt}")
                    nc.vector.tensor_add(lg[:tw], pl[:tw], rb2t[:tw])
                    m1 = op.tile([128, 1], f32, tag="m1", bufs=2,
                                 name=f"m1_{mt}")
                    nc.vector.reduce_max(m1[:tw], lg[:tw],
                                         axis=mybir.AxisListType.X)
                    nm1 = op.tile([128, 1], f32, tag="nm1", bufs=2,
                                  name=f"nm1_{mt}")
                    nc.vector.tensor_scalar_mul(nm1[:tw], m1[:tw], -1.0)
                    ismax = op.tile([128, E], f32, tag="ismax", bufs=2,
                                    name=f"ismax{mt}")
                    nc.vector.tensor_scalar(ismax[:tw], lg[:tw], m1[:tw],
                                            None, ALU.is_ge)
                    nc.vector.tensor_scalar_mul(ismax[:tw], ismax[:tw], -1e30)
                    nc.vector.tensor_add(ismax[:tw], ismax[:tw], lg[:tw])
                    t2 = op.tile([128, 1], f32, tag="t2", bufs=2,
                                 name=f"t2_{mt}")
                    nc.vector.reduce_max(t2[:tw], ismax[:tw],
                                         axis=mybir.AxisListType.X)
                    w_all = op.tile([128, E], f32, tag="w_all", bufs=2,
                                    name=f"w_all{mt}")
                    nc.vector.tensor_scalar(w_all[:tw], lg[:tw], t2[:tw],
                                            None, ALU.is_ge)
                    enum = op.tile([128, E], f32, tag="enum", bufs=2,
                                   name=f"enum{mt}")
                    nc.scalar.activation(enum[:tw], lg[:tw], AF.Exp,
                                         bias=nm1[:tw, 0:1])
                    den = op.tile([128, 1], f32, tag="den", bufs=2,
                                  name=f"den{mt}")
                    nc.scalar.activation(den[:tw], t2[:tw], AF.Exp,
                                         bias=nm1[:tw, 0:1])
                    nc.vector.tensor_scalar_add(den[:tw], den[:tw], 1.0)
                    rden = op.tile([128, 1], f32, tag="rden", bufs=2,
                                   name=f"rden{mt}")
                    nc.vector.reciprocal(rden[:tw], den[:tw])
                    nc.vector.tensor_mul(w_all[:tw], w_all[:tw], enum[:tw])
                    nc.vector.tensor_mul(w_all[:tw], w_all[:tw], selt[:tw])
                    wn = op.tile([128, 1], f32, tag="wn", bufs=2,
                                 name=f"wn{mt}")
                    nc.vector.reduce_sum(wn[:tw], w_all[:tw],
                                         axis=mybir.AxisListType.X)
                    nc.vector.tensor_scalar(
                        wmy[:tw, mt:mt + 1], wn[:tw], rden[:tw], None,
                        ALU.mult)

            # ---- expert matmul 1: ehT[m] = relu(eW1[m-tile].T @ xg + b) ----
            for m in range(KC2):
                if m == 5:
                    emit_router()
                wt = wts[m]
                if m + 6 < KC2:
                    load_ew1(m + 6)
                pss = [pp.tile([128, nw], f32, tag="mm1", bufs=4,
                               name=f"ps1_{m}_{ni}")
                       for ni, (ns, nw) in enumerate(NTL)]
                if m < 2:
                    # k-outer during the DMA ramp
                    for k in range(KC1):
                        for ni, (ns, nw) in enumerate(NTL):
                            nc.tensor.matmul(
                                pss[ni][:],
                                wt[:, k * 128:(k + 1) * 128],
                                xk[k][:, ns:ns + nw],
                                start=(k == 0),
                                stop=(k == KC1 - 1),
                            )
                else:
                    for ni, (ns, nw) in enumerate(NTL):
                        for k in range(KC1):
                            nc.tensor.matmul(
                                pss[ni][:],
                                wt[:, k * 128:(k + 1) * 128],
                                xk[k][:, ns:ns + nw],
                                start=(k == 0),
                                stop=(k == KC1 - 1),
                            )
                for ni, (ns, nw) in enumerate(NTL):
                    nc.scalar.activation(
                        ehs[m][:, ns:ns + nw], pss[ni][:],
                        AF.Relu, bias=eb1t[:, m:m + 1],
                    )

            # ---- expert matmul 2 + weighted combine ------------------------
            for mt, (ts, tw) in enumerate(TTL):
                po = pp.tile([128, NCLS], f32, tag="po", bufs=2,
                             name=f"po{mt}")
                for k2 in range(KC2):
                    nc.tensor.matmul(
                        po[:tw],
                        ehs[k2][:, ts:ts + tw],
                        ew2t[:, k2, :],
                        start=(k2 == 0),
                        stop=(k2 == KC2 - 1),
                    )
                osb = op.tile([128, NCLS], f32, tag="osb", bufs=3,
                              name=f"osb{mt}")
                nc.vector.tensor_add(osb[:tw], po[:tw], eb2t[:tw])
                nc.vector.tensor_scalar(
                    osb[:tw], osb[:tw], wmy[:tw, mt:mt + 1], None, ALU.mult,
                )
                nc.sync.dma_start(d_out[ts:ts + tw, :], osb[:tw])

    return nc


def _get_program(U):
    nc = _PROGRAMS.get(U)
    if nc is None:
        nc = _build_program(U)
        nc.finalize()
        _PROGRAMS[U] = nc
    return nc


def _dispatch_plan(xf, rW1, rb1, rW2, rb2):
    """Host-side sharding decision: top-2 token lists per expert (fp64
    router; device recomputes the router for the actual weights)."""
    rh = np.maximum(xf.astype(np.float64) @ np.asarray(rW1, np.float64)
                    + np.asarray(rb1, np.float64), 0.0)
    lg = rh @ np.asarray(rW2, np.float64) + np.asarray(rb2, np.float64)
    order = np.argsort(-lg, axis=1)
    top2 = order[:, :TOP_K]
    toks = []
    for e in range(E):
        toks.append(np.nonzero((top2 == e).any(axis=1))[0])
    return toks


def _prep_inputs(x, rW1, rb1, rW2, rb2, eW1, eb1, eW2, eb2):
    """Host-side shard/layout prep. Returns (U, in_maps, col_tokens)."""
    xf = np.ascontiguousarray(x.reshape(B, DIN), dtype=np.float32)
    toks = _dispatch_plan(xf, rW1, rb1, rW2, rb2)
    maxn = max(len(t) for t in toks)
    U = max(256, -(-maxn // 16) * 16)  # pad to 16 cols

    # [i, k, n] layout with hi/lo bf16 split
    xt = xf.reshape(B, KC1, 128).transpose(2, 1, 0)
    xh = xt.astype(BF16)
    xl = (xt - xh.astype(np.float32)).astype(BF16)

    w1 = np.asarray(rW1, np.float32).reshape(KC1, 128, RHID).transpose(1, 0, 2)
    w1h = np.ascontiguousarray(w1.astype(BF16))
    w1l = np.ascontiguousarray((w1 - w1h.astype(np.float32)).astype(BF16))

    rw2f = np.asarray(rW2, np.float32)
    rw2h = rw2f.astype(BF16)
    rw2l = np.ascontiguousarray((rw2f - rw2h.astype(np.float32)).astype(BF16))
    rw2h = np.ascontiguousarray(rw2h)
    rb1c = np.ascontiguousarray(np.asarray(rb1, np.float32).reshape(RHID, 1))
    rb2t = np.ascontiguousarray(
        np.tile(np.asarray(rb2, np.float32).reshape(1, E), (128, 1)))

    in_maps = []
    col_tokens = []
    member = np.zeros((E, B), bool)
    for e in range(E):
        member[e, toks[e]] = True
    for e in range(E):
        te = toks[e]
        # pad with tokens NOT routed to e -> their on-device w is 0
        pad_tok = int(np.nonzero(~member[e])[0][0])
        cols = np.concatenate(
            [te, np.full(U - len(te), pad_tok, dtype=te.dtype)])
        col_tokens.append(te)
        xgh = np.ascontiguousarray(xh[:, :, cols])
        xgl = np.ascontiguousarray(xl[:, :, cols])
        ew1 = np.ascontiguousarray(
            np.asarray(eW1[e], np.float32)
            .reshape(KC1, 128, KC2, 128)
            .transpose(2, 1, 0, 3)
            .reshape(KC2, 128, DIN)
            .astype(BF16)
        )
        ew2 = np.ascontiguousarray(
            np.asarray(eW2[e], np.float32)
            .reshape(KC2, 128, NCLS)
            .transpose(1, 0, 2)
            .astype(BF16)
        )
        eb1t = np.ascontiguousarray(
            np.asarray(eb1[e], np.float32).reshape(KC2, 128).T
        )
        eb2r = np.ascontiguousarray(
            np.tile(np.asarray(eb2[e], np.float32).reshape(1, NCLS), (128, 1))
        )
        sel = np.zeros((128, E), np.float32)
        sel[:, e] = 1.0
        in_maps.append({
            "xh": xgh, "xl": xgl,
            "w1h": w1h, "w1l": w1l,
            "rw2h": rw2h, "rw2l": rw2l, "rb1": rb1c, "rb2t": rb2t,
            "ew1": ew1, "ew2": ew2, "eb1": eb1t, "eb2t": eb2r,
            "sel": sel,
        })
    return U, in_maps, col_tokens


def kernel(x, rW1, rb1, rW2, rb2, eW1, eb1, eW2, eb2):
    global LAST_RESULTS
    _ensure_axon_profile_hook()
    from concourse.bass_utils import run_bass_kernel_spmd

    U, in_maps, col_tokens = _prep_inputs(
        x, rW1, rb1, rW2, rb2, eW1, eb1, eW2, eb2)
    nc = _get_program(U)
    res = run_bass_kernel_spmd(nc, in_maps, core_ids=list(range(E)))
    LAST_RESULTS = res
    out = np.zeros((B, NCLS), np.float32)
    for e, r in enumerate(res.results):
        part = np.asarray(r["out"], np.float32)
        out[col_tokens[e]] += part[:len(col_tokens[e])]
    return out


# revision 18
# speedup vs baseline: 2.1332x; 1.3874x over previous
"""Trainium2 Bass kernel for the MoE routing module (nn_MoE_53042846105633).

Strategy: expert-parallel with top-2 token dispatch, per the sharding hint
("all-to-all dispatch of tokens by top-k expert id").  The host computes the
dispatch PLAN (which tokens go to which expert's core) from an fp64 router
pass -- that is the sharding decision; full_io=true means the host mediates
all input distribution anyway.  All model numerics still run on device: each
core re-computes the router (bf16x2, 3-pass) over its gathered columns to
produce the top-2 softmax weights, runs its expert MLP slices over only
those columns, and emits w * (partial expert output).  The host scatter-adds
the 8 partial outputs by token id.

Work is balanced across cores with a uniform SPMD box template (same
instruction shapes on all cores; bindings are pure data):
  - AB box: 16 hid-chunk slots x W1 token-columns (one expert piece)
  - C  box: 4  hid-chunk slots x W2 token-columns (a quarter-chunk piece)
A big expert token-splits across several AB boxes; a small expert's 16
chunks can ride 4 C boxes on 4 cores; partial hid-chunk outputs sum on the
host.  Pad columns hold tokens NOT routed to the bound expert, so the
device's own (logit >= top2-threshold) mask zeroes their weight; their sel
rows are zero too.

Router precision: min top2/top3 logit gap on this data is 1.45e-4, far
below single-bf16 logit error (~4e-3), so both router matmuls run as bf16x2
(hi/lo split; logit error ~1e-5).  The expert MLP runs plain bf16 with fp32
accumulate (~3e-3 relative output error).  Collectives are deliberately
avoided: a NEFF with collectives runs the PE at 2.0 GHz instead of 2.4.
"""

import sys

sys.path.insert(0, "/opt/trn_rl_repo")

import numpy as np
import ml_dtypes

BF16 = ml_dtypes.bfloat16

# Model dims (fixed for this problem)
B = 1024          # tokens
DIN = 3072        # input features
RHID = 128        # router hidden
E = 8             # experts = cores
EHID = 2048       # expert hidden
NCLS = 10         # classes
TOP_K = 2
KC1 = DIN // 128  # 24 K-chunks for DIN contraction
KC2 = EHID // 128 # 16 K-chunks for EHID contraction

_PROGRAMS = {}
LAST_RESULTS = None


def _ensure_axon_profile_hook():
    """bass_utils' trace=True path imports antenv.axon_hooks, which this
    image lacks. Provide it (backed by libaxon_pjrt.so's NRT profile C API)
    so NTFF profiling works; degrade silently if unavailable."""
    import contextlib
    import ctypes
    import os
    import types

    try:
        from antenv.axon_hooks import get_axon_ntff_profile_hook  # noqa: F401
        return
    except ImportError:
        pass
    try:
        import antenv
    except ImportError:
        return

    state = {"hook": None}
    mod = types.ModuleType("antenv.axon_hooks")
    mod.set_axon_ntff_profile_hook = lambda h: state.__setitem__("hook", h)
    mod.get_axon_ntff_profile_hook = lambda: state["hook"]
    sys.modules["antenv.axon_hooks"] = mod
    antenv.axon_hooks = mod

    so_path = "/opt/axon/libaxon_pjrt.so"
    if not os.path.exists(so_path):
        return
    try:
        lib = ctypes.CDLL(so_path)
    except OSError:
        return
    if not hasattr(lib, "axon_start_nrt_profile"):
        return
    lib.axon_start_nrt_profile.argtypes = [
        ctypes.POINTER(ctypes.c_int64), ctypes.c_size_t]
    lib.axon_start_nrt_profile.restype = ctypes.c_int64
    lib.axon_stop_nrt_profile.argtypes = [ctypes.c_char_p]
    lib.axon_stop_nrt_profile.restype = ctypes.c_int64

    @contextlib.contextmanager
    def _hook(output_dir, device_ids):
        import jax

        jax.devices()
        if device_ids:
            ids = (ctypes.c_int64 * len(device_ids))(*device_ids)
            rc = lib.axon_start_nrt_profile(ids, len(device_ids))
        else:
            rc = lib.axon_start_nrt_profile(None, 0)
        if rc != 0:
            raise RuntimeError(f"axon_start_nrt_profile rc={rc}")
        try:
            yield
        finally:
            n = lib.axon_stop_nrt_profile(str(output_dir).encode())
            print(f"profile: {n} ntff file(s) -> {output_dir}",
                  file=sys.stderr)

    state["hook"] = _hook


def _tiles(total, step):
    return [(s, min(step, total - s)) for s in range(0, total, step)]


def _pad8(v):
    return -(-int(v) // 8) * 8


# ---------------------------------------------------------------------------
# Plan fitting: choose (W1, W2) and per-expert patterns via a tiny DP.
# Patterns (per expert, N tokens, 16 chunks):
#   AB1 : one AB box (N <= W1)
#   AB2 : two AB boxes, tokens split W1 + (N - W1) (N <= 2*W1)
#   ABC : one AB box (W1 tokens) + remainder in 4 C boxes (N - W1 <= W2)
#   C4  : all 16 chunks via 4 C boxes (N <= W2)
# ---------------------------------------------------------------------------

def _fit_plan(counts):
    counts = [int(c) for c in counts]
    nE = len(counts)
    maxc = max(counts)

    def _pad32(v):
        return -(-int(v) // 32) * 32

    # W1 must be 32-aligned so mm2 PSUM partition splits land on 32
    cands1 = sorted({_pad32(-(-c // k)) for c in counts for k in (1, 2, 3)
                     if c > 0} | {_pad32(maxc)})

    best = None
    for W1 in cands1:
        if W1 < 16:
            continue
        w2c = {0}
        for c in counts:
            if c <= W1:
                w2c.add(_pad8(c))
            if 0 < c - W1 <= W1:
                w2c.add(_pad8(c - W1))
        for W2 in sorted(w2c):
            if W2 > W1:
                continue
            # DP over experts, state = (ab_used, c_quads_used)
            INF = 10 ** 9
            dp = {(0, 0): (0, None)}
            for ei in range(nE):
                N = counts[ei]
                opts = []
                if N <= W1:
                    opts.append(("AB1", 1, 0))
                if W1 < N <= 2 * W1:
                    opts.append(("AB2", 2, 0))
                if W2 and 0 < N - W1 <= W2:
                    opts.append(("ABC", 1, 1))
                if W2 and N <= W2:
                    opts.append(("C4", 0, 1))
                if not opts:
                    dp = {}
                    break
                ndp = {}
                for (ab, cq), (cost, _) in dp.items():
                    for pat, dab, dcq in opts:
                        nab, ncq = ab + dab, cq + dcq
                        if nab > 8 or ncq > 2:
                            continue
                        key = (nab, ncq)
                        if key not in ndp or ndp[key][0] > cost:
                            ndp[key] = (cost, (ab, cq, pat))
                dp = ndp
            ok = [k for k in dp if k[0] <= 8 and k[1] <= 2]
            if not ok:
                continue
            cost = 16 * W1 + 4 * W2 + 0.08 * (W1 + W2) * 24 * 3  # mm1+router
            if best is None or cost < best[0]:
                best = (cost, W1, W2)
    if best is None:
        W1 = _pad32(maxc)
        return {"W1": W1, "W2": 0, "pats": ["AB1"] * nE}

    # Re-run the DP for the chosen (W1, W2) keeping backpointers.
    _, W1, W2 = best
    dp = {(0, 0): []}
    for ei in range(nE):
        N = counts[ei]
        opts = []
        if N <= W1:
            opts.append(("AB1", 1, 0))
        if W1 < N <= 2 * W1:
            opts.append(("AB2", 2, 0))
        if W2 and 0 < N - W1 <= W2:
            opts.append(("ABC", 1, 1))
        if W2 and N <= W2:
            opts.append(("C4", 0, 1))
        ndp = {}
        for (ab, cq), hist in dp.items():
            for pat, dab, dcq in opts:
                key = (ab + dab, cq + dcq)
                if key[0] > 8 or key[1] > 2:
                    continue
                if key not in ndp:
                    ndp[key] = hist + [pat]
        dp = ndp
    ok = sorted(dp)  # prefer fewer boxes
    pats = dp[ok[0]]
    return {"W1": W1, "W2": W2, "pats": pats}


def _make_boxes(counts, plan):
    """Expand patterns into AB / C box bindings.

    AB box: (expert, tok_lo, tok_hi, add_b2)
    C box:  (expert, chunk_lo, tok_lo, tok_hi, add_b2) or None (dummy)
    """
    W1, W2 = plan["W1"], plan["W2"]
    ab, cq = [], []
    for e, pat in enumerate(plan["pats"]):
        N = counts[e]
        # add_b2: the piece that covers chunk 0 for its token range adds b2
        if pat == "AB1":
            ab.append((e, 0, N, True))
        elif pat == "AB2":
            ab.append((e, 0, W1, True))
            ab.append((e, W1, N, True))
        elif pat == "ABC":
            ab.append((e, 0, W1, True))
            for ci in range(4):
                cq.append((e, 4 * ci, W1, N, ci == 0))
        elif pat == "C4":
            for ci in range(4):
                cq.append((e, 4 * ci, 0, N, ci == 0))
    while len(ab) < 8:
        ab.append(None)
    while len(cq) < 8:
        cq.append(None)
    return ab, cq


def _build_program(W1, W2a):
    """Uniform SPMD program: AB box (16 x W1) + optional C box (4 x W2a).

    Column layout: [C region: 0..W2a) then [AB region: W2a..W2a+W1).  W2a is
    a multiple of 128 so every 128-token tile lies in exactly one region --
    all mm2 PSUM writes start at partition 0 (HW requires base-partition 0
    for >32-partition matmul outputs).
    """
    import concourse.tile as tile
    from concourse import bacc, mybir

    f32 = mybir.dt.float32
    bf = mybir.dt.bfloat16
    AF = mybir.ActivationFunctionType
    ALU = mybir.AluOpType

    U = W1 + W2a
    SLOTS = 16 + (4 if W2a else 0)
    PSW = max(W1, W2a)  # shared mm1/router PSUM tile width
    # (slot, box_col_lo, box_width) per slot, in processing order (AB first)
    slot_geo = [(s, W2a, W1) for s in range(16)]
    if W2a:
        slot_geo += [(16 + i, 0, W2a) for i in range(4)]
    # router n-tiles (<= PSW wide so they share the PSUM pool tag)
    NTL = ([(0, W2a)] if W2a else []) + [(W2a, W1)]
    TTL = _tiles(U, 128)   # token tiles for logits / mm2
    NT = len(TTL)

    nc = bacc.Bacc("TRN2", debug=False, num_devices=E)

    d_xh = nc.dram_tensor("xh", [128, KC1, U], bf, kind="ExternalInput")
    d_xl = nc.dram_tensor("xl", [128, KC1, U], bf, kind="ExternalInput")
    d_w1h = nc.dram_tensor("w1h", [128, KC1, RHID], bf, kind="ExternalInput")
    d_w1l = nc.dram_tensor("w1l", [128, KC1, RHID], bf, kind="ExternalInput")
    d_rw2h = nc.dram_tensor("rw2h", [RHID, E], bf, kind="ExternalInput")
    d_rw2l = nc.dram_tensor("rw2l", [RHID, E], bf, kind="ExternalInput")
    d_rb1 = nc.dram_tensor("rb1", [RHID, 1], f32, kind="ExternalInput")
    d_rb2t = nc.dram_tensor("rb2t", [128, E], f32, kind="ExternalInput")
    d_ew1 = nc.dram_tensor("ew1", [SLOTS, 128, DIN], bf, kind="ExternalInput")
    d_ew2 = nc.dram_tensor("ew2", [128, SLOTS, NCLS], bf,
                           kind="ExternalInput")
    d_eb1 = nc.dram_tensor("eb1", [128, SLOTS], f32, kind="ExternalInput")
    d_b2r = nc.dram_tensor("b2r", [128, NT, NCLS], f32, kind="ExternalInput")
    d_sel = nc.dram_tensor("sel", [128, NT, E], f32, kind="ExternalInput")
    d_out = nc.dram_tensor("out", [U, NCLS], f32, kind="ExternalOutput")

    with tile.TileContext(nc) as tc:
        with (
            tc.tile_pool(name="const", bufs=1) as cp,
            tc.tile_pool(name="wstream", bufs=8) as wp,
            tc.tile_pool(name="psum", bufs=1, space="PSUM") as pp,
            tc.tile_pool(name="outp", bufs=1) as op,
        ):
            # ---- HAM pre-warm: flip clock gate to 2.4 GHz while DMA ramps --
            warmt = cp.tile([128, 128], bf, tag="warmt", name="warmt")
            nc.vector.memset(warmt[:], 1.0)
            warm = pp.tile([128, 128], f32, tag="po", bufs=2, name="warm")
            for _i in range(44):
                nc.tensor.matmul(warm[:], warmt[:], warmt[:],
                                 start=True, stop=True)

            # ---- input DMA (emission order ~= DMA queue order) -------------
            wts = {}

            def load_ew1(s):
                wt = wp.tile([128, DIN], bf, tag="ew1", name=f"ew1s{s}")
                nc.sync.dma_start(wt[:, :DIN // 2], d_ew1[s][:, :DIN // 2])
                nc.sync.dma_start(wt[:, DIN // 2:], d_ew1[s][:, DIN // 2:])
                wts[s] = wt

            xk = []
            for k in range(KC1):
                t = cp.tile([128, U], bf, tag=f"xk{k}", name=f"xk{k}")
                xk.append(t)
            nc.sync.dma_start(xk[0][:], d_xh[:, 0, :])
            load_ew1(0)
            for k in range(1, KC1):
                nc.sync.dma_start(xk[k][:], d_xh[:, k, :])
            for _s in range(1, 8):
                load_ew1(_s)
            eb1t = cp.tile([128, SLOTS], f32, tag="eb1", name="eb1t")
            nc.sync.dma_start(eb1t[:], d_eb1[:])
            rb1t = cp.tile([RHID, 1], f32, tag="rb1", name="rb1t")
            nc.sync.dma_start(rb1t[:], d_rb1[:])
            w1ht = cp.tile([128, KC1, RHID], bf, tag="w1h", name="w1ht")
            nc.sync.dma_start(w1ht[:], d_w1h[:])
            w1lt = cp.tile([128, KC1, RHID], bf, tag="w1l", name="w1lt")
            nc.sync.dma_start(w1lt[:], d_w1l[:])
            xlk = []
            for k in range(KC1):
                t = cp.tile([128, U], bf, tag=f"xlk{k}", name=f"xlk{k}")
                nc.sync.dma_start(t[:], d_xl[:, k, :])
                xlk.append(t)
            ew2t = cp.tile([128, SLOTS, NCLS], bf, tag="ew2", name="ew2t")
            nc.sync.dma_start(ew2t[:], d_ew2[:])
            b2rt = cp.tile([128, NT, NCLS], f32, tag="b2r", name="b2rt")
            nc.sync.dma_start(b2rt[:], d_b2r[:])
            selt = cp.tile([128, NT, E], f32, tag="sel", name="selt")
            nc.sync.dma_start(selt[:], d_sel[:])
            rw2ht = cp.tile([RHID, E], bf, tag="rw2h", name="rw2ht")
            nc.sync.dma_start(rw2ht[:], d_rw2h[:])
            rw2lt = cp.tile([RHID, E], bf, tag="rw2l", name="rw2lt")
            nc.sync.dma_start(rw2lt[:], d_rw2l[:])
            rb2t = cp.tile([128, E], f32, tag="rb2t", name="rb2t")
            nc.sync.dma_start(rb2t[:], d_rb2t[:])

            # eh per slot: relu(eW1_slot.T @ xg) in [hid, tok] layout, bf16
            ehs = [cp.tile([128, wdt], bf, tag=f"eh{s}", name=f"eh{s}")
                   for s, lo, wdt in slot_geo]

            wmy = cp.tile([128, NT], f32, tag="wmy", name="wmy")

            def emit_router():
                rh = cp.tile([RHID, U], f32, tag="rh", name="rh")
                for ns, nw in NTL:
                    psr = pp.tile([128, PSW], f32, tag="mm1", bufs=4,
                                  name=f"psr{ns}")
                    passes = [(w1ht, xk), (w1ht, xlk), (w1lt, xk)]
                    for pi, (wt_, xs_) in enumerate(passes):
                        for k in range(KC1):
                            nc.tensor.matmul(
                                psr[:, :nw],
                                wt_[:, k, :],
                                xs_[k][:, ns:ns + nw],
                                start=(pi == 0 and k == 0),
                                stop=(pi == 2 and k == KC1 - 1),
                            )
                    nc.scalar.activation(
                        rh[:, ns:ns + nw], psr[:, :nw],
                        AF.Relu, bias=rb1t[:, 0:1],
                    )
                rhh = cp.tile([RHID, U], bf, tag="rhh", name="rhh")
                nc.vector.tensor_copy(rhh[:], rh[:])
                rhl = cp.tile([RHID, U], bf, tag="rhl", name="rhl")
                nc.vector.tensor_sub(rhl[:], rh[:], rhh[:])

                # logits + top-2 weight per token tile; for the column's
                # expert e: w = exp(l_e - m1) * (l_e >= t2) / (1 + exp(t2-m1))
                for mt, (ts, tw) in enumerate(TTL):
                    tsl = slice(ts, ts + tw)
                    pl = pp.tile([128, E], f32, tag="lg", bufs=2,
                                 name=f"pl{mt}")
                    nc.tensor.matmul(pl[:tw], rhh[:, tsl], rw2ht[:],
                                     start=True, stop=False)
                    nc.tensor.matmul(pl[:tw], rhh[:, tsl], rw2lt[:],
                                     start=False, stop=False)
                    nc.tensor.matmul(pl[:tw], rhl[:, tsl], rw2ht[:],
                                     start=False, stop=True)
                    lg = op.tile([128, E], f32, tag="lg_sb", bufs=2,
                                 name=f"lg{mt}")
                    nc.vector.tensor_add(lg[:tw], pl[:tw], rb2t[:tw])
                    m1 = op.tile([128, 1], f32, tag="m1", bufs=2,
                                 name=f"m1_{mt}")
                    nc.vector.reduce_max(m1[:tw], lg[:tw],
                                         axis=mybir.AxisListType.X)
                    nm1 = op.tile([128, 1], f32, tag="nm1", bufs=2,
                                  name=f"nm1_{mt}")
                    nc.vector.tensor_scalar_mul(nm1[:tw], m1[:tw], -1.0)
                    ismax = op.tile([128, E], f32, tag="ismax", bufs=2,
                                    name=f"ismax{mt}")
                    nc.vector.tensor_scalar(ismax[:tw], lg[:tw], m1[:tw],
                                            None, ALU.is_ge)
                    nc.vector.tensor_scalar_mul(ismax[:tw], ismax[:tw], -1e30)
                    nc.vector.tensor_add(ismax[:tw], ismax[:tw], lg[:tw])
                    t2 = op.tile([128, 1], f32, tag="t2", bufs=2,
                                 name=f"t2_{mt}")
                    nc.vector.reduce_max(t2[:tw], ismax[:tw],
                                         axis=mybir.AxisListType.X)
                    w_all = op.tile([128, E], f32, tag="w_all", bufs=2,
                                    name=f"w_all{mt}")
                    nc.vector.tensor_scalar(w_all[:tw], lg[:tw], t2[:tw],
                                            None, ALU.is_ge)
                    enum = op.tile([128, E], f32, tag="enum", bufs=2,
                                   name=f"enum{mt}")
                    nc.scalar.activation(enum[:tw], lg[:tw], AF.Exp,
                                         bias=nm1[:tw, 0:1])
                    den = op.tile([128, 1], f32, tag="den", bufs=2,
                                  name=f"den{mt}")
                    nc.scalar.activation(den[:tw], t2[:tw], AF.Exp,
                                         bias=nm1[:tw, 0:1])
                    nc.vector.tensor_scalar_add(den[:tw], den[:tw], 1.0)
                    rden = op.tile([128, 1], f32, tag="rden", bufs=2,
                                   name=f"rden{mt}")
                    nc.vector.reciprocal(rden[:tw], den[:tw])
                    nc.vector.tensor_mul(w_all[:tw], w_all[:tw], enum[:tw])
                    nc.vector.tensor_mul(w_all[:tw], w_all[:tw],
                                         selt[:tw, mt, :])
                    wn = op.tile([128, 1], f32, tag="wn", bufs=2,
                                 name=f"wn{mt}")
                    nc.vector.reduce_sum(wn[:tw], w_all[:tw],
                                         axis=mybir.AxisListType.X)
                    nc.vector.tensor_scalar(
                        wmy[:tw, mt:mt + 1], wn[:tw], rden[:tw], None,
                        ALU.mult)

            # ---- expert matmul 1 per slot ----------------------------------
            for si, (s, lo, wdt) in enumerate(slot_geo):
                if si == 5:
                    emit_router()
                wt = wts[s]
                if si + 8 < SLOTS:
                    load_ew1(si + 8)
                ps = pp.tile([128, PSW], f32, tag="mm1", bufs=4,
                             name=f"ps1_{s}")
                for k in range(KC1):
                    nc.tensor.matmul(
                        ps[:, :wdt],
                        wt[:, k * 128:(k + 1) * 128],
                        xk[k][:, lo:lo + wdt],
                        start=(k == 0),
                        stop=(k == KC1 - 1),
                    )
                nc.scalar.activation(
                    ehs[si][:], ps[:, :wdt],
                    AF.Relu, bias=eb1t[:, s:s + 1],
                )

            # ---- expert matmul 2 + weighted combine ------------------------
            for mt, (ts, tw) in enumerate(TTL):
                po = pp.tile([128, NCLS], f32, tag="po", bufs=2,
                             name=f"po{mt}")
                # contributions: slots whose box overlaps [ts, ts+tw)
                by_range = {}
                for si, (s, lo, wdt) in enumerate(slot_geo):
                    o1, o2 = max(ts, lo), min(ts + tw, lo + wdt)
                    if o1 >= o2:
                        continue
                    by_range.setdefault((o1, o2, lo), []).append(si)
                for (o1, o2, lo), sis in by_range.items():
                    p1, p2 = o1 - ts, o2 - ts
                    for j, si in enumerate(sis):
                        s = slot_geo[si][0]
                        nc.tensor.matmul(
                            po[p1:p2],
                            ehs[si][:, o1 - lo:o2 - lo],
                            ew2t[:, s, :],
                            start=(j == 0),
                            stop=(j == len(sis) - 1),
                        )
                osb = op.tile([128, NCLS], f32, tag="osb", bufs=3,
                              name=f"osb{mt}")
                nc.vector.tensor_add(osb[:tw], po[:tw], b2rt[:tw, mt, :])
                nc.vector.tensor_scalar(
                    osb[:tw], osb[:tw], wmy[:tw, mt:mt + 1], None, ALU.mult,
                )
                nc.sync.dma_start(d_out[ts:ts + tw, :], osb[:tw])

    return nc


def _get_program(W1, W2):
    key = (W1, W2)
    nc = _PROGRAMS.get(key)
    if nc is None:
        nc = _build_program(W1, W2)
        nc.finalize()
        _PROGRAMS[key] = nc
    return nc


def _dispatch_plan(xf, rW1, rb1, rW2, rb2):
    """Host-side sharding decision: top-2 token lists per expert (fp64
    router; device recomputes the router for the actual weights)."""
    rh = np.maximum(xf.astype(np.float64) @ np.asarray(rW1, np.float64)
                    + np.asarray(rb1, np.float64), 0.0)
    lg = rh @ np.asarray(rW2, np.float64) + np.asarray(rb2, np.float64)
    order = np.argsort(-lg, axis=1)
    top2 = order[:, :TOP_K]
    toks = []
    for e in range(E):
        toks.append(np.nonzero((top2 == e).any(axis=1))[0])
    return toks


def _prep_inputs(x, rW1, rb1, rW2, rb2, eW1, eb1, eW2, eb2):
    xf = np.ascontiguousarray(x.reshape(B, DIN), dtype=np.float32)
    toks = _dispatch_plan(xf, rW1, rb1, rW2, rb2)
    counts = [len(t) for t in toks]
    plan = _fit_plan(counts)
    W1, W2 = plan["W1"], plan["W2"]
    W2a = -(-W2 // 128) * 128 if W2 else 0   # C region padded to 128
    ab_boxes, c_boxes = _make_boxes(counts, plan)
    U = W1 + W2a
    SLOTS = 16 + (4 if W2a else 0)
    NT = len(_tiles(U, 128))

    xt = xf.reshape(B, KC1, 128).transpose(2, 1, 0)
    xh = xt.astype(BF16)
    xl = (xt - xh.astype(np.float32)).astype(BF16)

    w1 = np.asarray(rW1, np.float32).reshape(KC1, 128, RHID).transpose(1, 0, 2)
    w1h = np.ascontiguousarray(w1.astype(BF16))
    w1l = np.ascontiguousarray((w1 - w1h.astype(np.float32)).astype(BF16))

    rw2f = np.asarray(rW2, np.float32)
    rw2h = rw2f.astype(BF16)
    rw2l = np.ascontiguousarray((rw2f - rw2h.astype(np.float32)).astype(BF16))
    rw2h = np.ascontiguousarray(rw2h)
    rb1c = np.ascontiguousarray(np.asarray(rb1, np.float32).reshape(RHID, 1))
    rb2t = np.ascontiguousarray(
        np.tile(np.asarray(rb2, np.float32).reshape(1, E), (128, 1)))

    member = np.zeros((E, B), bool)
    for e in range(E):
        member[e, toks[e]] = True

    # per-expert device layouts (built once, sliced per box)
    ew1_l = {}
    ew2_l = {}
    eb1_l = {}
    for e in range(E):
        ew1_l[e] = (np.asarray(eW1[e], np.float32)
                    .reshape(KC1, 128, KC2, 128)
                    .transpose(2, 1, 0, 3)
                    .reshape(KC2, 128, DIN)
                    .astype(BF16))
        ew2_l[e] = (np.asarray(eW2[e], np.float32)
                    .reshape(KC2, 128, NCLS)
                    .transpose(1, 0, 2)
                    .astype(BF16))
        eb1_l[e] = np.asarray(eb1[e], np.float32).reshape(KC2, 128).T

    in_maps = []
    core_places = []   # per core: list of (row_lo, token_ids)
    for core in range(E):
        pieces = []    # (expert|None, chunk_lo, nchunks, col_lo, tok_ids)
        abx = ab_boxes[core]
        if abx is not None:
            e, t0, t1, ab_b2 = abx
            pieces.append((e, 0, 16, W2a, toks[e][t0:t1], ab_b2))
        else:
            pieces.append((None, 0, 16, W2a, np.empty(0, np.int64), False))
        if W2a:
            cbx = c_boxes[core]
            if cbx is not None:
                e, ch_lo, t0, t1, c_b2 = cbx
                pieces.append((e, ch_lo, 4, 0, toks[e][t0:t1], c_b2))
            else:
                pieces.append((None, 0, 4, 0, np.empty(0, np.int64), False))

        cols = np.zeros(U, np.int64)
        selc = np.zeros((U, E), np.float32)
        b2c = np.zeros((U, NCLS), np.float32)
        ew1c = np.zeros((SLOTS, 128, DIN), BF16)
        ew2c = np.zeros((128, SLOTS, NCLS), BF16)
        eb1c = np.zeros((128, SLOTS), np.float32)
        places = []
        slot0 = 0
        for (e, ch_lo, nch, col_lo, tids, add_b2) in pieces:
            wbox = W1 if col_lo == W2a else W2a
            if e is not None:
                n = len(tids)
                pad_tok = int(np.nonzero(~member[e])[0][0])
                cols[col_lo:col_lo + n] = tids
                cols[col_lo + n:col_lo + wbox] = pad_tok
                selc[col_lo:col_lo + n, e] = 1.0
                if add_b2:
                    b2c[col_lo:col_lo + n, :] = np.asarray(
                        eb2[e], np.float32).reshape(1, NCLS)
                ew1c[slot0:slot0 + nch] = ew1_l[e][ch_lo:ch_lo + nch]
                ew2c[:, slot0:slot0 + nch, :] = \
                    ew2_l[e][:, ch_lo:ch_lo + nch, :]
                eb1c[:, slot0:slot0 + nch] = eb1_l[e][:, ch_lo:ch_lo + nch]
                places.append((col_lo, tids))
            slot0 += nch

        xgh = np.ascontiguousarray(xh[:, :, cols])
        xgl = np.ascontiguousarray(xl[:, :, cols])
        # sel/b2 in [128, NT, *] tile layout
        sel3 = np.zeros((128, NT, E), np.float32)
        b2r3 = np.zeros((128, NT, NCLS), np.float32)
        for mt, (ts, tw) in enumerate(_tiles(U, 128)):
            sel3[:tw, mt, :] = selc[ts:ts + tw]
            b2r3[:tw, mt, :] = b2c[ts:ts + tw]

        in_maps.append({
            "xh": xgh, "xl": xgl,
            "w1h": w1h, "w1l": w1l,
            "rw2h": rw2h, "rw2l": rw2l, "rb1": rb1c, "rb2t": rb2t,
            "ew1": np.ascontiguousarray(ew1c),
            "ew2": np.ascontiguousarray(ew2c),
            "eb1": np.ascontiguousarray(eb1c),
            "b2r": np.ascontiguousarray(b2r3),
            "sel": np.ascontiguousarray(sel3),
        })
        core_places.append(places)
    return W1, W2a, in_maps, core_places


def kernel(x, rW1, rb1, rW2, rb2, eW1, eb1, eW2, eb2):
    global LAST_RESULTS
    _ensure_axon_profile_hook()
    from concourse.bass_utils import run_bass_kernel_spmd

    W1, W2a, in_maps, core_places = _prep_inputs(
        x, rW1, rb1, rW2, rb2, eW1, eb1, eW2, eb2)
    nc = _get_program(W1, W2a)
    res = run_bass_kernel_spmd(nc, in_maps, core_ids=list(range(E)))
    LAST_RESULTS = res
    out = np.zeros((B, NCLS), np.float32)
    for core, r in enumerate(res.results):
        part = np.asarray(r["out"], np.float32)
        for (col_lo, tids) in core_places[core]:
            out[tids] += part[col_lo:col_lo + len(tids)]
    return out
